# revision 38
# baseline (speedup 1.0000x reference)
"""Sliding-window causal self-attention (B=2, T=2048, D=1024, H=16, dk=64, W=512)
on 8 Trainium2 NeuronCores.

Sharding: core = (b, hg) for b in {0,1}, head-group hg in {0..3}.
Data parallel over batch, tensor parallel over heads: each core gets
x[b]^T, the 4-head column slices of Wq/Wk/Wv (+bq slice) and the matching
row slice of Wo, and produces a partial [T, D] output.  Host gathers with
out[b] = sum_hg partial[b,hg] + (bv @ Wo + bo).

Math notes (exact softmax identities, validated vs reference):
 - bk shifts every logit of a row by a per-row constant -> cancels in softmax.
 - bv enters the output linearly with weights summing to 1 -> folded into the
   host-side bias term bv @ Wo (+ bo), added once after the cross-core sum.
 - no max-subtraction in softmax: logits are O(1), fp32 exp is safe.

All matmul operands are bf16 (1 cycle/row at any moving width, lower PE power
than fp32r -> avoids the DVFS throttle fp32r hits); accumulation stays fp32 in
PSUM.  Inputs are pre-cast and pre-laid-out on the host so every DMA lands as
contiguous [128, X] blocks (4-8KB per partition).  Output partials are written
bf16 and summed in fp32 on the host.

Device algorithm per core (fully unrolled Tile kernel):
  Q^T = Wq_c^T x^T + bq_c  [256, 2048] (lhsT = Wq k-chunks, rhs = x^T)
  K^T = Wk_c^T x^T         [256, 2048]
  V   = x Wv_c             [2048, 4 heads x (64 + ones column)]
  x^T is streamed in four 512-column blocks (2 SBUF slots), and the four
  projection column-blocks are interleaved with the first head-pair's
  attention to keep the PE dense.
  per head h, per key-block J (128 keys), band i-j in [0, 511]:
    S^T[j, i] = K_h^T J-block (stationary, 64-row contraction) @ Q_h^T over
                the i-window [J*128, J*128+640) clipped to T
                (psum [128, 640], two matmuls N=512 + N=128)
    P^T = exp(0.125 * S^T) in one ACT op -> SBUF bf16; two static 128x128
          triangular masks (host inputs) zero the out-of-band corners
          (applied on GPSIMD / DVE).
  per 4-query-block group g (512 queries), per head:
    O_aug^T [65, 512] accumulates V_aug (stationary [128, 65]) @ P^T slices
    over the 8 contributing key-blocks (one closed psum accumulation group,
    full-width contribution first because start=True zeroes the whole bank);
    row 64 accumulates the softmax denominators.
    normalize: denom row -> SBUF (ACT), rank-1 ones-matmul broadcast to
    [64, 512] (PE), reciprocal_approx_fast (custom DVE), multiply -> O_hat^T.
  after the last head of a group: out rows = O_hat^T chunks (stationary)
  @ Wo_c -> [128, 1024] psum pair, staged (GPSIMD+DVE) and DMA'd out bf16,
  so the output transfer overlaps the remaining attention work.
"""

import math
from contextlib import ExitStack

import numpy as np
import ml_dtypes

import concourse.bass as bass
import concourse.mybir as mybir
import concourse.tile as tile
from concourse import bacc
from concourse.bass_utils import run_bass_kernel_spmd

F32 = mybir.dt.float32
F32R = mybir.dt.float32r
BF16 = mybir.dt.bfloat16
NPBF = ml_dtypes.bfloat16

T = 2048
D = 1024
NHEAD = 16
DK = 64
WINDOW = 512
HPC = 4            # heads per core
HCOLS = HPC * DK   # 256 projected columns per core
NJ = T // 128      # 16 j/query blocks
NKC = D // 128     # 8 contraction chunks over D
NG = 4             # query-block groups of 512

_NC_CACHE = {}


def _emit(tc):
    nc = tc.nc
    xT_d = nc.dram_tensor("xT", [128, 4, NKC, 512], BF16, kind="ExternalInput").ap()
    wq_d = nc.dram_tensor("wq", [128, NKC, HCOLS], BF16, kind="ExternalInput").ap()
    wk_d = nc.dram_tensor("wk", [128, NKC, HCOLS], BF16, kind="ExternalInput").ap()
    wv_d = nc.dram_tensor("wv", [128, NKC, HCOLS], BF16, kind="ExternalInput").ap()
    wo_d = nc.dram_tensor("wo", [128, 2, D], BF16, kind="ExternalInput").ap()
    bq_d = nc.dram_tensor("bqp", [128, 2], F32, kind="ExternalInput").ap()
    mlo_d = nc.dram_tensor("mlo", [128, 128], BF16, kind="ExternalInput").ap()
    mhi_d = nc.dram_tensor("mhi", [128, 128], BF16, kind="ExternalInput").ap()
    on2_d = nc.dram_tensor("on2", [33, 128], F32R, kind="ExternalInput").ap()
    zer_d = nc.dram_tensor("zer", [32, 512], F32R, kind="ExternalInput").ap()
    out_d = nc.dram_tensor("out", [T, D], BF16, kind="ExternalOutput").ap()

    with ExitStack() as ctx:
        const_pool = ctx.enter_context(tc.tile_pool(name="const", bufs=1))
        qk_pool = ctx.enter_context(tc.tile_pool(name="qk", bufs=1))
        w_pool = ctx.enter_context(tc.tile_pool(name="w", bufs=1))
        xt_pool = ctx.enter_context(tc.tile_pool(name="xt", bufs=2))
        pt_pool = ctx.enter_context(tc.tile_pool(name="pt", bufs=16))
        nrm_pool = ctx.enter_context(tc.tile_pool(name="nrm", bufs=3))
        stage_pool = ctx.enter_context(tc.tile_pool(name="stage", bufs=2))
        ps_s = ctx.enter_context(tc.tile_pool(name="ps_s", bufs=2, space="PSUM"))
        ps_pv = ctx.enter_context(tc.tile_pool(name="ps_pv", bufs=2, space="PSUM"))
        ps_mi = ctx.enter_context(tc.tile_pool(name="ps_mi", bufs=2, space="PSUM"))

        bq_sb = const_pool.tile([128, 2], F32)
        mask_lo = const_pool.tile([128, 128], BF16)   # keep c >= p (upper incl)
        mask_hi = const_pool.tile([128, 128], BF16)   # keep c < p (strict lower)
        # head-pair denominator broadcast: dens live at partitions 0 and 32
        # (engine writes must start at a 32-aligned partition); the [33,128]
        # selector maps row 0 -> out rows 0-63, row 32 -> out rows 64-127
        ones_sel = const_pool.tile([33, 128], F32R)
        den33 = {0: const_pool.tile([33, 512], F32R, name="den33a"),
                 1: const_pool.tile([33, 512], F32R, name="den33b")}

        wo_sb = qk_pool.tile([128, 2, D], BF16)
        # V storage [j-part, J, head, dk+1]; col 64 of each head slot = 1.0
        v_sb = qk_pool.tile([128, NJ, HPC, DK + 1], BF16)
        q_sb = qk_pool.tile([128, 2, T], BF16)
        k_sb = qk_pool.tile([128, 2, T], BF16)
        osb = qk_pool.tile([128, 2, T], BF16)   # normalized O^T

        wq_sb = w_pool.tile([128, NKC, HCOLS], BF16)
        wk_sb = w_pool.tile([128, NKC, HCOLS], BF16)
        wv_sb = w_pool.tile([128, NKC, HCOLS], BF16)

        # ---- x^T streamed by 512-column blocks with slot reuse (2 live) ----
        xt_tiles = {}

        def xt_dma(cb):
            xt_tiles[cb] = xt_pool.tile([128, NKC, 512], BF16, tag="xt",
                                        name=f"xt_c{cb}")
            nc.sync.dma_start(xt_tiles[cb][:, 0:4, :], xT_d[:, cb, 0:4, :])
            nc.gpsimd.dma_start(xt_tiles[cb][:, 4:8, :], xT_d[:, cb, 4:8, :])

        # startup: half-tensor DMA granularity (2-4KB packets), issued in PE
        # consumption order round-robined over the three DMA-capable queues
        # so cumulative arrival tracks cumulative demand
        nc.sync.dma_start(bq_sb[:], bq_d[:, :])
        nc.vector.memset(
            v_sb[:, :, :, DK:DK + 1].rearrange("p j h o -> p (j h o)"), 1.0)
        xt_tiles[0] = xt_pool.tile([128, NKC, 512], BF16, tag="xt", name="xt_c0")
        xt_tiles[1] = xt_pool.tile([128, NKC, 512], BF16, tag="xt", name="xt_c1")
        nc.sync.dma_start(wq_sb[:, 0:4, :], wq_d[:, 0:4, :])
        nc.gpsimd.dma_start(wq_sb[:, 4:8, :], wq_d[:, 4:8, :])
        nc.sync.dma_start(xt_tiles[0][:, :, :], xT_d[:, 0, :, :])
        nc.scalar.dma_start(wk_sb[:, 0:4, :], wk_d[:, 0:4, :])
        nc.scalar.dma_start(wk_sb[:, 4:8, :], wk_d[:, 4:8, :])
        nc.gpsimd.dma_start(xt_tiles[1][:, :, :], xT_d[:, 1, :, :])
        nc.scalar.dma_start(wv_sb[:, :, :], wv_d[:, :, :])
        nc.sync.dma_start(mask_lo[:], mlo_d[:, :])
        nc.sync.dma_start(mask_hi[:], mhi_d[:, :])
        nc.sync.dma_start(ones_sel[:], on2_d[:, :])
        nc.sync.dma_start(den33[0][0:32, :], zer_d[:, :])
        nc.sync.dma_start(den33[1][0:32, :], zer_d[:, :])
        nc.scalar.dma_start(wo_sb[:, :, :], wo_d[:, :, :])

        def proj_qk(cb, ms):
            """Q^T/K^T m-chunks of `ms` for one 512-column block of x."""
            xt = xt_tiles[cb]
            nsl = slice(cb * 512, (cb + 1) * 512)
            for m in ms:
                qp = ps_mi.tile([128, 512], F32, tag="mi", name=f"qp{cb}{m}")
                for k in range(NKC):
                    nc.tensor.matmul(
                        qp[:], wq_sb[:, k, m * 128:(m + 1) * 128],
                        xt[:, k, :], start=(k == 0), stop=(k == NKC - 1),
                    )
                nc.scalar.activation(
                    q_sb[:, m, nsl], qp[:],
                    mybir.ActivationFunctionType.Identity,
                    bias=bq_sb[:, m:m + 1],
                )
                kp = ps_mi.tile([128, 512], F32, tag="mi", name=f"kp{cb}{m}")
                for k in range(NKC):
                    nc.tensor.matmul(
                        kp[:], wk_sb[:, k, m * 128:(m + 1) * 128],
                        xt[:, k, :], start=(k == 0), stop=(k == NKC - 1),
                    )
                nc.vector.tensor_copy(k_sb[:, m, nsl], kp[:])

        def proj_v(cb):
            xt = xt_tiles[cb]
            for r in range(4 * cb, 4 * cb + 4):
                vp = ps_mi.tile([128, HPC, DK], F32, tag="mi", name=f"vp{r}")
                for k in range(NKC):
                    nc.tensor.matmul(
                        vp[:], xt[:, k, (r % 4) * 128:(r % 4) * 128 + 128],
                        wv_sb[:, k, :], start=(k == 0), stop=(k == NKC - 1),
                    )
                nc.vector.tensor_copy(v_sb[:, r, :, 0:DK], vp[:])

        def attn_j(hpair, pt_tiles, J, hi_eng=None):
            width = min(640, T - J * 128)
            wA = min(512, width)
            wB = width - wA
            for part in range(2):           # row-group-alternating A then B
                for h in hpair:
                    hp = slice((h % 2) * 64, (h % 2) * 64 + 64)
                    hc = h // 2
                    if part == 0:
                        pt = pt_pool.tile([128, 640], BF16, tag="pt",
                                          name=f"pt_h{h}_J{J}")
                        pt_tiles[h][J] = pt
                        s = ps_s.tile([128, 640], F32, tag="s",
                                      name=f"s_h{h}_J{J}")
                        pt_tiles[h][(J, "s")] = s
                        nc.tensor.matmul(
                            s[:, 0:wA], k_sb[hp, hc, J * 128:(J + 1) * 128],
                            q_sb[hp, hc, J * 128:J * 128 + wA],
                            start=True, stop=True,
                        )
                    else:
                        s = pt_tiles[h].pop((J, "s"))
                        pt = pt_tiles[h][J]
                        if wB > 0:
                            nc.tensor.matmul(
                                s[:, 512:512 + wB],
                                k_sb[hp, hc, J * 128:(J + 1) * 128],
                                q_sb[hp, hc, J * 128 + 512:J * 128 + width],
                                start=True, stop=True,
                            )
                        nc.scalar.activation(
                            pt[:, 0:width], s[:, 0:width],
                            mybir.ActivationFunctionType.Exp, scale=0.125,
                        )
                        nc.gpsimd.tensor_mul(pt[:, 0:128], pt[:, 0:128],
                                             mask_lo[:])
                        if width == 640:
                            (hi_eng or nc.gpsimd).tensor_mul(
                                pt[:, 512:640], pt[:, 512:640], mask_hi[:])

        def attn_group(hpair, pt_tiles, g):
            g0 = 512 * g
            pvs = {}
            for h in hpair:
                pv = ps_pv.tile([65, 512], F32, tag="pv", name=f"pv_h{h}_g{g}")
                pvs[h] = pv
                jps = []
                for Jp in range(max(0, 4 * g - 4), 4 * g + 4):
                    wJp = min(640, T - Jp * 128)
                    lo = max(Jp * 128, g0)
                    hi = min(Jp * 128 + wJp, g0 + 512)
                    if hi > lo:
                        jps.append((Jp, lo, hi))
                # start=True lazily zeroes the whole psum bank; a full-width
                # contribution must come first
                jps.sort(key=lambda t: -(t[2] - t[1]))
                assert jps[0][2] - jps[0][1] == 512
                for idx, (Jp, lo, hi) in enumerate(jps):
                    nc.tensor.matmul(
                        pv[:, lo - g0:hi - g0],
                        v_sb[:, Jp, h, :],
                        pt_tiles[h][Jp][:, lo - Jp * 128:hi - Jp * 128],
                        start=(idx == 0), stop=(idx == len(jps) - 1),
                    )
                for Jp in range(max(0, 4 * g - 4), 4 * g):
                    pt_tiles[h].pop(Jp, None)

            # normalize both heads at once: dens -> partitions 0/32 of the
            # pair's den33 (on the vector queue, which is idle at group time;
            # the scalar queue is backlogged with exps), one selector matmul
            # broadcasts to [128,512], one reciprocal, per-head mul into osb
            hc = hpair[0] // 2
            d33 = den33[hc]
            # pair01 groups: vector is busy with k/v casts -> use scalar;
            # pair23 groups: scalar is backlogged with exps -> use vector
            if hc == 0:
                nc.scalar.copy(d33[0:1, :], pvs[hpair[0]][64:65, :])
                nc.scalar.copy(d33[32:33, :], pvs[hpair[1]][64:65, :])
            else:
                nc.vector.tensor_copy(d33[0:1, :], pvs[hpair[0]][64:65, :])
                nc.vector.tensor_copy(d33[32:33, :], pvs[hpair[1]][64:65, :])
            bcp = ps_mi.tile([128, 512], F32, tag="mi", name=f"bcp_p{hc}_g{g}")
            nc.tensor.matmul(bcp[:], ones_sel[:], d33[:],
                             start=True, stop=True)
            rcp = nrm_pool.tile([128, 512], F32, tag="rcp",
                                name=f"rcp_p{hc}_g{g}")
            nc.vector.reciprocal_approx_fast(rcp[:], bcp[:])
            for i, h in enumerate(hpair):
                hp = slice((h % 2) * 64, (h % 2) * 64 + 64)
                nc.vector.tensor_mul(
                    osb[hp, hc, g0:g0 + 512], pvs[h][0:64, :],
                    rcp[i * 64:(i + 1) * 64, :],
                )

            if hpair[-1] == HPC - 1:   # all heads complete: output projection
                # staging alternates scalar/vector so the two po psum banks
                # recycle without waiting on one queue; each half DMAs out
                # as soon as it is staged
                for qb in range(4 * g, 4 * g + 4):
                    so = stage_pool.tile([128, 1024], BF16, tag="stage",
                                         name=f"so{qb}")
                    for nh in range(2):
                        po = ps_mi.tile([128, 512], F32, tag="mi",
                                        name=f"po{qb}_{nh}")
                        for c in range(2):
                            nc.tensor.matmul(
                                po[:], osb[:, c, qb * 128:(qb + 1) * 128],
                                wo_sb[:, c, nh * 512:(nh + 1) * 512],
                                start=(c == 0), stop=(c == 1),
                            )
                        sl = slice(nh * 512, (nh + 1) * 512)
                        if nh == 0:
                            nc.scalar.copy(so[:, sl], po[:])
                        else:
                            nc.vector.tensor_copy(so[:, sl], po[:])
                    # alternate issue queues so the final drain isn't
                    # serialized on one engine's DMA ring
                    deng = nc.sync if qb % 2 == 0 else nc.gpsimd
                    deng.dma_start(out_d[qb * 128:(qb + 1) * 128, :], so[:, :])

        # ---- schedule: proj c0/c1, then pair01 attention with proj c2/c3
        # and x^T DMA for c2/c3 interleaved, then pair23 attention + Wo ----
        pt01 = {0: {}, 1: {}}
        pt23 = {2: {}, 3: {}}
        proj_qk(0, (0, 1))
        proj_qk(1, (0, 1))
        # early S/exp for pair01 J=0-2 fills the PE while x/wv DMAs land
        for J in range(3):
            attn_j((0, 1), pt01, J, hi_eng=nc.vector)
        proj_v(0)
        proj_v(1)
        xt_dma(2)
        for J in range(NJ):
            if J >= 3:
                attn_j((0, 1), pt01, J, hi_eng=nc.vector)
            if J % 4 == 3:
                attn_group((0, 1), pt01, J // 4)
            if J == 3:
                proj_qk(2, (0,))   # pair01 J=4 reads q/k m0 up to col 1408
                proj_v(2)
                xt_dma(3)
            elif J == 7:
                proj_qk(3, (0,))   # pair01 J=8 reads q/k m0 up to col 1664
                proj_v(3)
            elif J == 11:
                proj_qk(2, (1,))   # heads 2-3 projections fill the pair01 tail
            elif J == 15:
                proj_qk(3, (1,))
        for J in range(NJ):
            attn_j((2, 3), pt23, J)
            if J % 4 == 3:
                attn_group((2, 3), pt23, J // 4)


def _build():
    if "nc" in _NC_CACHE:
        return _NC_CACHE["nc"]
    nc = bacc.Bacc("TRN2", debug=False)
    with tile.TileContext(nc) as tc:
        _emit(tc)
    nc.compile()
    _NC_CACHE["nc"] = nc
    return nc


def _shard_inputs(x, Wq, bq, Wk, Wv, Wo):
    idx = np.arange(128)
    mlo = (idx[None, :] >= idx[:, None]).astype(NPBF)  # c >= p
    mhi = (idx[None, :] < idx[:, None]).astype(NPBF)   # c < p
    on2 = np.zeros((33, 128), np.float32)
    on2[0, 0:64] = 1.0
    on2[32, 64:128] = 1.0
    in_maps = []
    for b in range(2):
        # [128p, 4cb, 8k, 512j] with [p,cb,k,j] = x[b, cb*512+j, k*128+p]
        xT = np.ascontiguousarray(
            x[b].astype(NPBF).reshape(4, 512, NKC, 128).transpose(3, 0, 2, 1))
        for hg in range(4):
            cols = slice(hg * HCOLS, (hg + 1) * HCOLS)
            in_maps.append({
                "xT": xT,
                "wq": np.ascontiguousarray(
                    Wq[:, cols].astype(NPBF).reshape(NKC, 128, HCOLS)
                    .transpose(1, 0, 2)),
                "wk": np.ascontiguousarray(
                    Wk[:, cols].astype(NPBF).reshape(NKC, 128, HCOLS)
                    .transpose(1, 0, 2)),
                "wv": np.ascontiguousarray(
                    Wv[:, cols].astype(NPBF).reshape(NKC, 128, HCOLS)
                    .transpose(1, 0, 2)),
                "wo": np.ascontiguousarray(
                    Wo[cols, :].astype(NPBF).reshape(2, 128, D)
                    .transpose(1, 0, 2)),
                "bqp": np.ascontiguousarray(bq[cols].reshape(2, 128).T),
                "mlo": mlo, "mhi": mhi, "on2": on2,
                "zer": np.zeros((32, 512), np.float32),
            })
    return in_maps


def kernel(x, Wq, bq, Wk, bk, Wv, bv, Wo, bo, _trace=False, _tmpdir=None):
    x = np.asarray(x, dtype=np.float32)
    Wq = np.asarray(Wq, dtype=np.float32)
    Wk = np.asarray(Wk, dtype=np.float32)
    Wv = np.asarray(Wv, dtype=np.float32)
    Wo = np.asarray(Wo, dtype=np.float32)
    bq = np.asarray(bq, dtype=np.float32)
    bv = np.asarray(bv, dtype=np.float32)
    bo = np.asarray(bo, dtype=np.float32)

    nc = _build()
    in_maps = _shard_inputs(x, Wq, bq, Wk, Wv, Wo)
    res = run_bass_kernel_spmd(
        nc, in_maps, core_ids=list(range(8)), trace=_trace, tmpdir=_tmpdir,
    )
    host_bias = (bv @ Wo + bo).astype(np.float32)
    out = np.zeros((2, T, D), dtype=np.float32)
    for b in range(2):
        acc = res.results[b * 4]["out"].astype(np.float32).copy()
        for hg in range(1, 4):
            acc += res.results[b * 4 + hg]["out"].astype(np.float32)
        out[b] = acc + host_bias
    kernel._last_results = res
    return out


# revision 41
# speedup vs baseline: 1.0024x; 1.0024x over previous
"""Sliding-window causal self-attention (B=2, T=2048, D=1024, H=16, dk=64, W=512)
on 8 Trainium2 NeuronCores.

Sharding: core = (b, hg) for b in {0,1}, head-group hg in {0..3}.
Data parallel over batch, tensor parallel over heads: each core gets
x[b]^T, the 4-head column slices of Wq/Wk/Wv (+bq slice) and the matching
row slice of Wo, and produces a partial [T, D] output.  Host gathers with
out[b] = sum_hg partial[b,hg] + (bv @ Wo + bo).

Math notes (exact softmax identities, validated vs reference):
 - bk shifts every logit of a row by a per-row constant -> cancels in softmax.
 - bv enters the output linearly with weights summing to 1 -> folded into the
   host-side bias term bv @ Wo (+ bo), added once after the cross-core sum.
 - no max-subtraction in softmax: logits are O(1), fp32 exp is safe.

All matmul operands are bf16 (1 cycle/row at any moving width, lower PE power
than fp32r -> avoids the DVFS throttle fp32r hits); accumulation stays fp32 in
PSUM.  Inputs are pre-cast and pre-laid-out on the host so every DMA lands as
contiguous [128, X] blocks (4-8KB per partition).  Output partials are written
bf16 and summed in fp32 on the host.

Device algorithm per core (fully unrolled Tile kernel):
  Q^T = Wq_c^T x^T + bq_c  [256, 2048] (lhsT = Wq k-chunks, rhs = x^T)
  K^T = Wk_c^T x^T         [256, 2048]
  V   = x Wv_c             [2048, 4 heads x (64 + ones column)]
  x^T is streamed in four 512-column blocks (2 SBUF slots), and the four
  projection column-blocks are interleaved with the first head-pair's
  attention to keep the PE dense.
  per head h, per key-block J (128 keys), band i-j in [0, 511]:
    S^T[j, i] = K_h^T J-block (stationary, 64-row contraction) @ Q_h^T over
                the i-window [J*128, J*128+640) clipped to T
                (psum [128, 640], two matmuls N=512 + N=128)
    P^T = exp(0.125 * S^T) in one ACT op -> SBUF bf16; two static 128x128
          triangular masks (host inputs) zero the out-of-band corners
          (applied on GPSIMD / DVE).
  per 4-query-block group g (512 queries), per head:
    O_aug^T [65, 512] accumulates V_aug (stationary [128, 65]) @ P^T slices
    over the 8 contributing key-blocks (one closed psum accumulation group,
    full-width contribution first because start=True zeroes the whole bank);
    row 64 accumulates the softmax denominators.
    normalize: denom row -> SBUF (ACT), rank-1 ones-matmul broadcast to
    [64, 512] (PE), reciprocal_approx_fast (custom DVE), multiply -> O_hat^T.
  after the last head of a group: out rows = O_hat^T chunks (stationary)
  @ Wo_c -> [128, 1024] psum pair, staged (GPSIMD+DVE) and DMA'd out bf16,
  so the output transfer overlaps the remaining attention work.
"""

import math
from contextlib import ExitStack

import numpy as np
import ml_dtypes

import concourse.bass as bass
import concourse.mybir as mybir
import concourse.tile as tile
from concourse import bacc
from concourse.bass_utils import run_bass_kernel_spmd

F32 = mybir.dt.float32
F32R = mybir.dt.float32r
BF16 = mybir.dt.bfloat16
NPBF = ml_dtypes.bfloat16

T = 2048
D = 1024
NHEAD = 16
DK = 64
WINDOW = 512
HPC = 4            # heads per core
HCOLS = HPC * DK   # 256 projected columns per core
NJ = T // 128      # 16 j/query blocks
NKC = D // 128     # 8 contraction chunks over D
NG = 4             # query-block groups of 512

_NC_CACHE = {}


def _emit(tc):
    nc = tc.nc
    xT_d = nc.dram_tensor("xT", [128, 4, NKC, 512], BF16, kind="ExternalInput").ap()
    wq_d = nc.dram_tensor("wq", [128, NKC, HCOLS], BF16, kind="ExternalInput").ap()
    wk_d = nc.dram_tensor("wk", [128, NKC, HCOLS], BF16, kind="ExternalInput").ap()
    wv_d = nc.dram_tensor("wv", [128, NKC, HCOLS], BF16, kind="ExternalInput").ap()
    wo_d = nc.dram_tensor("wo", [128, 2, D], BF16, kind="ExternalInput").ap()
    bq_d = nc.dram_tensor("bqp", [128, 2], F32, kind="ExternalInput").ap()
    mlo_d = nc.dram_tensor("mlo", [128, 128], BF16, kind="ExternalInput").ap()
    mhi_d = nc.dram_tensor("mhi", [128, 128], BF16, kind="ExternalInput").ap()
    on2_d = nc.dram_tensor("on2", [33, 128], F32R, kind="ExternalInput").ap()
    zer_d = nc.dram_tensor("zer", [32, 512], F32R, kind="ExternalInput").ap()
    out_d = nc.dram_tensor("out", [T, D], BF16, kind="ExternalOutput").ap()

    with ExitStack() as ctx:
        const_pool = ctx.enter_context(tc.tile_pool(name="const", bufs=1))
        qk_pool = ctx.enter_context(tc.tile_pool(name="qk", bufs=1))
        w_pool = ctx.enter_context(tc.tile_pool(name="w", bufs=1))
        xt_pool = ctx.enter_context(tc.tile_pool(name="xt", bufs=2))
        pt_pool = ctx.enter_context(tc.tile_pool(name="pt", bufs=16))
        nrm_pool = ctx.enter_context(tc.tile_pool(name="nrm", bufs=3))
        stage_pool = ctx.enter_context(tc.tile_pool(name="stage", bufs=2))
        ps_s = ctx.enter_context(tc.tile_pool(name="ps_s", bufs=2, space="PSUM"))
        ps_pv = ctx.enter_context(tc.tile_pool(name="ps_pv", bufs=2, space="PSUM"))
        ps_mi = ctx.enter_context(tc.tile_pool(name="ps_mi", bufs=2, space="PSUM"))

        bq_sb = const_pool.tile([128, 2], F32)
        mask_lo = const_pool.tile([128, 128], BF16)   # keep c >= p (upper incl)
        mask_hi = const_pool.tile([128, 128], BF16)   # keep c < p (strict lower)
        # head-pair denominator broadcast: dens live at partitions 0 and 32
        # (engine writes must start at a 32-aligned partition); the [33,128]
        # selector maps row 0 -> out rows 0-63, row 32 -> out rows 64-127
        ones_sel = const_pool.tile([33, 128], F32R)
        den33 = {0: const_pool.tile([33, 512], F32R, name="den33a"),
                 1: const_pool.tile([33, 512], F32R, name="den33b")}

        wo_sb = qk_pool.tile([128, 2, D], BF16)
        # V storage [j-part, J, head, dk+1]; col 64 of each head slot = 1.0
        v_sb = qk_pool.tile([128, NJ, HPC, DK + 1], BF16)
        q_sb = qk_pool.tile([128, 2, T], BF16)
        k_sb = qk_pool.tile([128, 2, T], BF16)
        osb = qk_pool.tile([128, 2, T], BF16)   # normalized O^T

        wq_sb = w_pool.tile([128, NKC, HCOLS], BF16)
        wk_sb = w_pool.tile([128, NKC, HCOLS], BF16)
        wv_sb = w_pool.tile([128, NKC, HCOLS], BF16)

        # ---- x^T streamed by 512-column blocks with slot reuse (2 live) ----
        xt_tiles = {}

        def xt_dma(cb):
            xt_tiles[cb] = xt_pool.tile([128, NKC, 512], BF16, tag="xt",
                                        name=f"xt_c{cb}")
            nc.sync.dma_start(xt_tiles[cb][:, 0:4, :], xT_d[:, cb, 0:4, :])
            nc.gpsimd.dma_start(xt_tiles[cb][:, 4:8, :], xT_d[:, cb, 4:8, :])

        # startup: half-tensor DMA granularity (2-4KB packets), issued in PE
        # consumption order round-robined over the three DMA-capable queues
        # so cumulative arrival tracks cumulative demand
        nc.sync.dma_start(bq_sb[:], bq_d[:, :])
        nc.vector.memset(
            v_sb[:, :, :, DK:DK + 1].rearrange("p j h o -> p (j h o)"), 1.0)
        xt_tiles[0] = xt_pool.tile([128, NKC, 512], BF16, tag="xt", name="xt_c0")
        xt_tiles[1] = xt_pool.tile([128, NKC, 512], BF16, tag="xt", name="xt_c1")
        nc.sync.dma_start(wq_sb[:, 0:4, :], wq_d[:, 0:4, :])
        nc.gpsimd.dma_start(wq_sb[:, 4:8, :], wq_d[:, 4:8, :])
        nc.scalar.dma_start(xt_tiles[0][:, 0:4, :], xT_d[:, 0, 0:4, :])
        nc.sync.dma_start(xt_tiles[0][:, 4:8, :], xT_d[:, 0, 4:8, :])
        nc.gpsimd.dma_start(wk_sb[:, 0:4, :], wk_d[:, 0:4, :])
        nc.scalar.dma_start(wk_sb[:, 4:8, :], wk_d[:, 4:8, :])
        nc.sync.dma_start(xt_tiles[1][:, 0:4, :], xT_d[:, 1, 0:4, :])
        nc.gpsimd.dma_start(xt_tiles[1][:, 4:8, :], xT_d[:, 1, 4:8, :])
        nc.scalar.dma_start(wv_sb[:, :, :], wv_d[:, :, :])
        nc.sync.dma_start(mask_lo[:], mlo_d[:, :])
        nc.sync.dma_start(mask_hi[:], mhi_d[:, :])
        nc.sync.dma_start(ones_sel[:], on2_d[:, :])
        nc.sync.dma_start(den33[0][0:32, :], zer_d[:, :])
        nc.sync.dma_start(den33[1][0:32, :], zer_d[:, :])
        nc.scalar.dma_start(wo_sb[:, :, :], wo_d[:, :, :])

        def proj_qk(cb, ms):
            """Q^T/K^T m-chunks of `ms` for one 512-column block of x."""
            xt = xt_tiles[cb]
            nsl = slice(cb * 512, (cb + 1) * 512)
            for m in ms:
                qp = ps_mi.tile([128, 512], F32, tag="mi", name=f"qp{cb}{m}")
                for k in range(NKC):
                    nc.tensor.matmul(
                        qp[:], wq_sb[:, k, m * 128:(m + 1) * 128],
                        xt[:, k, :], start=(k == 0), stop=(k == NKC - 1),
                    )
                nc.scalar.activation(
                    q_sb[:, m, nsl], qp[:],
                    mybir.ActivationFunctionType.Identity,
                    bias=bq_sb[:, m:m + 1],
                )
                kp = ps_mi.tile([128, 512], F32, tag="mi", name=f"kp{cb}{m}")
                for k in range(NKC):
                    nc.tensor.matmul(
                        kp[:], wk_sb[:, k, m * 128:(m + 1) * 128],
                        xt[:, k, :], start=(k == 0), stop=(k == NKC - 1),
                    )
                nc.scalar.copy(k_sb[:, m, nsl], kp[:])

        def proj_v(cb):
            xt = xt_tiles[cb]
            for r in range(4 * cb, 4 * cb + 4):
                vp = ps_mi.tile([128, HPC, DK], F32, tag="mi", name=f"vp{r}")
                for k in range(NKC):
                    nc.tensor.matmul(
                        vp[:], xt[:, k, (r % 4) * 128:(r % 4) * 128 + 128],
                        wv_sb[:, k, :], start=(k == 0), stop=(k == NKC - 1),
                    )
                nc.vector.tensor_copy(v_sb[:, r, :, 0:DK], vp[:])

        def attn_j(hpair, pt_tiles, J, hi_eng=None):
            width = min(640, T - J * 128)
            wA = min(512, width)
            wB = width - wA
            for part in range(2):           # row-group-alternating A then B
                for h in hpair:
                    hp = slice((h % 2) * 64, (h % 2) * 64 + 64)
                    hc = h // 2
                    if part == 0:
                        pt = pt_pool.tile([128, 640], BF16, tag="pt",
                                          name=f"pt_h{h}_J{J}")
                        pt_tiles[h][J] = pt
                        s = ps_s.tile([128, 640], F32, tag="s",
                                      name=f"s_h{h}_J{J}")
                        pt_tiles[h][(J, "s")] = s
                        nc.tensor.matmul(
                            s[:, 0:wA], k_sb[hp, hc, J * 128:(J + 1) * 128],
                            q_sb[hp, hc, J * 128:J * 128 + wA],
                            start=True, stop=True,
                        )
                    else:
                        s = pt_tiles[h].pop((J, "s"))
                        pt = pt_tiles[h][J]
                        if wB > 0:
                            nc.tensor.matmul(
                                s[:, 512:512 + wB],
                                k_sb[hp, hc, J * 128:(J + 1) * 128],
                                q_sb[hp, hc, J * 128 + 512:J * 128 + width],
                                start=True, stop=True,
                            )
                        nc.scalar.activation(
                            pt[:, 0:width], s[:, 0:width],
                            mybir.ActivationFunctionType.Exp, scale=0.125,
                        )
                        nc.gpsimd.tensor_mul(pt[:, 0:128], pt[:, 0:128],
                                             mask_lo[:])
                        if width == 640:
                            (hi_eng or nc.gpsimd).tensor_mul(
                                pt[:, 512:640], pt[:, 512:640], mask_hi[:])

        def attn_group(hpair, pt_tiles, g):
            g0 = 512 * g
            pvs = {}
            for h in hpair:
                pv = ps_pv.tile([65, 512], F32, tag="pv", name=f"pv_h{h}_g{g}")
                pvs[h] = pv
                jps = []
                for Jp in range(max(0, 4 * g - 4), 4 * g + 4):
                    wJp = min(640, T - Jp * 128)
                    lo = max(Jp * 128, g0)
                    hi = min(Jp * 128 + wJp, g0 + 512)
                    if hi > lo:
                        jps.append((Jp, lo, hi))
                # start=True lazily zeroes the whole psum bank; a full-width
                # contribution must come first
                jps.sort(key=lambda t: -(t[2] - t[1]))
                assert jps[0][2] - jps[0][1] == 512
                for idx, (Jp, lo, hi) in enumerate(jps):
                    nc.tensor.matmul(
                        pv[:, lo - g0:hi - g0],
                        v_sb[:, Jp, h, :],
                        pt_tiles[h][Jp][:, lo - Jp * 128:hi - Jp * 128],
                        start=(idx == 0), stop=(idx == len(jps) - 1),
                    )
                for Jp in range(max(0, 4 * g - 4), 4 * g):
                    pt_tiles[h].pop(Jp, None)

            # normalize both heads at once: dens -> partitions 0/32 of the
            # pair's den33 (on the vector queue, which is idle at group time;
            # the scalar queue is backlogged with exps), one selector matmul
            # broadcasts to [128,512], one reciprocal, per-head mul into osb
            hc = hpair[0] // 2
            d33 = den33[hc]
            # pair01 groups: vector is busy with k/v casts -> use scalar;
            # pair23 groups: scalar is backlogged with exps -> use vector
            if hc == 0:
                nc.scalar.copy(d33[0:1, :], pvs[hpair[0]][64:65, :])
                nc.scalar.copy(d33[32:33, :], pvs[hpair[1]][64:65, :])
            else:
                nc.vector.tensor_copy(d33[0:1, :], pvs[hpair[0]][64:65, :])
                nc.vector.tensor_copy(d33[32:33, :], pvs[hpair[1]][64:65, :])
            bcp = ps_mi.tile([128, 512], F32, tag="mi", name=f"bcp_p{hc}_g{g}")
            nc.tensor.matmul(bcp[:], ones_sel[:], d33[:],
                             start=True, stop=True)
            rcp = nrm_pool.tile([128, 512], F32, tag="rcp",
                                name=f"rcp_p{hc}_g{g}")
            nc.vector.reciprocal_approx_fast(rcp[:], bcp[:])
            for i, h in enumerate(hpair):
                hp = slice((h % 2) * 64, (h % 2) * 64 + 64)
                nc.vector.tensor_mul(
                    osb[hp, hc, g0:g0 + 512], pvs[h][0:64, :],
                    rcp[i * 64:(i + 1) * 64, :],
                )

            if hpair[-1] == HPC - 1:   # all heads complete: output projection
                # staging alternates scalar/vector so the two po psum banks
                # recycle without waiting on one queue; each half DMAs out
                # as soon as it is staged
                for qb in range(4 * g, 4 * g + 4):
                    so = stage_pool.tile([128, 1024], BF16, tag="stage",
                                         name=f"so{qb}")
                    for nh in range(2):
                        po = ps_mi.tile([128, 512], F32, tag="mi",
                                        name=f"po{qb}_{nh}")
                        for c in range(2):
                            nc.tensor.matmul(
                                po[:], osb[:, c, qb * 128:(qb + 1) * 128],
                                wo_sb[:, c, nh * 512:(nh + 1) * 512],
                                start=(c == 0), stop=(c == 1),
                            )
                        sl = slice(nh * 512, (nh + 1) * 512)
                        if nh == 0:
                            nc.scalar.copy(so[:, sl], po[:])
                        else:
                            nc.vector.tensor_copy(so[:, sl], po[:])
                    nc.sync.dma_start(
                        out_d[qb * 128:(qb + 1) * 128, :], so[:, :])

        # ---- schedule: proj c0/c1, then pair01 attention with proj c2/c3
        # and x^T DMA for c2/c3 interleaved, then pair23 attention + Wo ----
        pt01 = {0: {}, 1: {}}
        pt23 = {2: {}, 3: {}}
        proj_qk(0, (0, 1))
        proj_qk(1, (0, 1))
        # early S/exp for pair01 J=0-2 fills the PE while x/wv DMAs land
        for J in range(3):
            attn_j((0, 1), pt01, J, hi_eng=nc.vector)
        proj_v(0)
        proj_v(1)
        xt_dma(2)
        for J in range(NJ):
            if J >= 3:
                attn_j((0, 1), pt01, J, hi_eng=nc.vector)
            if J % 4 == 3:
                attn_group((0, 1), pt01, J // 4)
            if J == 3:
                proj_qk(2, (0,))   # pair01 J=4 reads q/k m0 up to col 1408
                proj_v(2)
                xt_dma(3)
            elif J == 7:
                proj_qk(3, (0,))   # pair01 J=8 reads q/k m0 up to col 1664
                proj_v(3)
            elif J == 11:
                proj_qk(2, (1,))   # heads 2-3 projections fill the pair01 tail
            elif J == 15:
                proj_qk(3, (1,))
        for J in range(NJ):
            attn_j((2, 3), pt23, J)
            if J % 4 == 3:
                attn_group((2, 3), pt23, J // 4)


def _build():
    if "nc" in _NC_CACHE:
        return _NC_CACHE["nc"]
    nc = bacc.Bacc("TRN2", debug=False)
    with tile.TileContext(nc) as tc:
        _emit(tc)
    nc.compile()
    _NC_CACHE["nc"] = nc
    return nc


def _shard_inputs(x, Wq, bq, Wk, Wv, Wo):
    idx = np.arange(128)
    mlo = (idx[None, :] >= idx[:, None]).astype(NPBF)  # c >= p
    mhi = (idx[None, :] < idx[:, None]).astype(NPBF)   # c < p
    on2 = np.zeros((33, 128), np.float32)
    on2[0, 0:64] = 1.0
    on2[32, 64:128] = 1.0
    in_maps = []
    for b in range(2):
        # [128p, 4cb, 8k, 512j] with [p,cb,k,j] = x[b, cb*512+j, k*128+p]
        xT = np.ascontiguousarray(
            x[b].astype(NPBF).reshape(4, 512, NKC, 128).transpose(3, 0, 2, 1))
        for hg in range(4):
            cols = slice(hg * HCOLS, (hg + 1) * HCOLS)
            in_maps.append({
                "xT": xT,
                "wq": np.ascontiguousarray(
                    Wq[:, cols].astype(NPBF).reshape(NKC, 128, HCOLS)
                    .transpose(1, 0, 2)),
                "wk": np.ascontiguousarray(
                    Wk[:, cols].astype(NPBF).reshape(NKC, 128, HCOLS)
                    .transpose(1, 0, 2)),
                "wv": np.ascontiguousarray(
                    Wv[:, cols].astype(NPBF).reshape(NKC, 128, HCOLS)
                    .transpose(1, 0, 2)),
                "wo": np.ascontiguousarray(
                    Wo[cols, :].astype(NPBF).reshape(2, 128, D)
                    .transpose(1, 0, 2)),
                "bqp": np.ascontiguousarray(bq[cols].reshape(2, 128).T),
                "mlo": mlo, "mhi": mhi, "on2": on2,
                "zer": np.zeros((32, 512), np.float32),
            })
    return in_maps


def kernel(x, Wq, bq, Wk, bk, Wv, bv, Wo, bo, _trace=False, _tmpdir=None):
    x = np.asarray(x, dtype=np.float32)
    Wq = np.asarray(Wq, dtype=np.float32)
    Wk = np.asarray(Wk, dtype=np.float32)
    Wv = np.asarray(Wv, dtype=np.float32)
    Wo = np.asarray(Wo, dtype=np.float32)
    bq = np.asarray(bq, dtype=np.float32)
    bv = np.asarray(bv, dtype=np.float32)
    bo = np.asarray(bo, dtype=np.float32)

    nc = _build()
    in_maps = _shard_inputs(x, Wq, bq, Wk, Wv, Wo)
    res = run_bass_kernel_spmd(
        nc, in_maps, core_ids=list(range(8)), trace=_trace, tmpdir=_tmpdir,
    )
    host_bias = (bv @ Wo + bo).astype(np.float32)
    out = np.zeros((2, T, D), dtype=np.float32)
    for b in range(2):
        acc = res.results[b * 4]["out"].astype(np.float32).copy()
        for hg in range(1, 4):
            acc += res.results[b * 4 + hg]["out"].astype(np.float32)
        out[b] = acc + host_bias
    kernel._last_results = res
    return out
